# revision 2
# baseline (speedup 1.0000x reference)
"""Trainium2 Bass kernel for the recurrent-column step (nn_Column_23398981829106).

Strategy (8 NeuronCores, width sharded in 256-row shards):
  - core m holds slab  = weights[256m:256m+256, :, :]      (natural layout)
            and slabT = weights[256m:256m+256, :, 1:8].T   (host pre-transposed, j-major)
  - einsum1 hidden[w,d] = sum_i W[i,w,d] * state[i,d]: each core contracts its
    own i-rows on the PE (masked-diagonal stationary operand accumulates all d
    into one [8, 512] PSUM per w-block) -> partial hidden^T [8, 2048]
    -> AllReduce -> each core slices its own 256 columns (partition_id).
  - einsum2 contrib[i,a] = sum_j W[i,j,a+1] * sg[j,a+1]: local and complete on
    slabT (contracts j on the PE partitions), [7, 256] PSUM.
  - circular buffers sa / og: stream shard through SBUF, overwrite the 9
    columns at t=time with state_new / output_weights, stream back out. The
    sg/act-gather columns are free strided reads from those SBUF tiles.
  - col = sg * relu_mask shards -> AllGather -> colT [8, 2048] via PE
    transposes -> flattened to [1, 16384] (PE operands must start at
    partition 0).
  - gradients[i, w, d] = act_prev[i,d] * col[w,d]: K=1 outer-product matmuls
    (lhsT = act_prev^T row d [1,128], rhs = colT row d chunk [1,256]) into
    [128, 2048] PSUM (d-major), interleaved to (w,d)-major during the
    PSUM->SBUF copy (alternating DVE/ACT), then full-rate 1MB DMAs out.

Host side shards the inputs, sums the scalar output and concatenates the
per-core output shards.
"""

import sys

sys.path.insert(0, "/opt/trn_rl_repo")

import numpy as np

import concourse.bass as bass
import concourse.bacc as bacc
import concourse.tile as tile
import concourse.mybir as mybir
from concourse import bass_utils

F32 = mybir.dt.float32
ALU = mybir.AluOpType

P = 128          # partitions
W = 2048         # column width
D = 8            # depth
DP1 = D + 1
MOD = 30 * D + 2  # 242
S = 256          # shard rows per core
NCORES = 8
CT = DP1 * MOD   # 2178 flattened (c, t) free dim of sa/og shards


def _runs(offsets):
    """Group an increasing offset list into (dst_start, src_start, count, step)
    strided runs (one engine copy each)."""
    out = []
    i = 0
    n = len(offsets)
    while i < n:
        if i == n - 1:
            out.append((i, offsets[i], 1, 1))
            break
        d = offsets[i + 1] - offsets[i]
        j = i + 1
        while j + 1 < n and offsets[j + 1] - offsets[j] == d:
            j += 1
        if d <= 0:
            out.append((i, offsets[i], 1, 1))
            i += 1
        else:
            out.append((i, offsets[i], j - i + 1, d))
            i = j + 1
    return out


def _gather(nc, dst, src, offsets):
    for d0, s0, cnt, step in _runs(offsets):
        nc.vector.tensor_copy(dst[:, d0 : d0 + cnt], src[:, bass.ds(s0, cnt, step)])


def _build(time):
    t = int(time)
    tidx = [(t - 2 * D + 2 * (a + 1)) % MOD for a in range(D - 1)]  # a = 0..6
    tia = [(t - 2 * D + 2 * (aa + 1)) % MOD for aa in range(D)]     # aa = 0..7

    rm_off = [(a + 2) * MOD + (tidx[a] + 1) % MOD for a in range(D - 1)]
    og_off = [(a + 1) * MOD + tidx[a] for a in range(D - 1)]
    ap_off = [aa * MOD + (tia[aa] - 1) % MOD for aa in range(D)]
    rg_off = [(aa + 1) * MOD + tia[aa] for aa in range(D - 1)]
    # none of these may alias the column being scattered at t (odd/even parity
    # guarantees it; assert for safety with arbitrary `time`)
    for off in rm_off + og_off + ap_off + rg_off:
        assert off % MOD != t, (time, off)

    nc = bacc.Bacc("TRN2", target_bir_lowering=False, debug=False, num_devices=NCORES)

    slab_d = nc.dram_tensor("slab", [2, P, W * D], F32, kind="ExternalInput")
    slabT_d = nc.dram_tensor("slabT", [16, P, S * (D - 1)], F32, kind="ExternalInput")
    stt_d = nc.dram_tensor("stt", [2, P, D], F32, kind="ExternalInput")
    xin_d = nc.dram_tensor("xin", [2, P, 1], F32, kind="ExternalInput")
    ow_d = nc.dram_tensor("ow", [2, P, DP1], F32, kind="ExternalInput")
    sgin_d = nc.dram_tensor("sgin", [16, P, D - 1], F32, kind="ExternalInput")
    ident_d = nc.dram_tensor("ident", [P, P], F32, kind="ExternalInput")
    sa_d = nc.dram_tensor("sa", [2, P, CT], F32, kind="ExternalInput")
    og_d = nc.dram_tensor("og", [2, P, CT], F32, kind="ExternalInput")

    o_state = nc.dram_tensor("o_state", [2, P, DP1], F32, kind="ExternalOutput")
    o_sg = nc.dram_tensor("o_sg", [2, P, D], F32, kind="ExternalOutput")
    o_sa = nc.dram_tensor("o_sa", [2, P, CT], F32, kind="ExternalOutput")
    o_og = nc.dram_tensor("o_og", [2, P, CT], F32, kind="ExternalOutput")
    o_gr = nc.dram_tensor("o_gr", [2, P, W * D], F32, kind="ExternalOutput")

    rg = [list(range(NCORES))]

    with tile.TileContext(nc) as tc:
        with tc.tile_pool(name="cp", bufs=1) as cp, \
             tc.tile_pool(name="dram", bufs=1, space="DRAM") as dram:
            # persistent small tiles
            ident = cp.tile([P, P], F32, name="ident")
            nc.scalar.dma_start(ident[:], ident_d[:])
            st_t, x_t, ow_t = [], [], []
            for p in range(2):
                st_p = cp.tile([P, D], F32, name=f"st{p}")
                x_p = cp.tile([P, 1], F32, name=f"x{p}")
                ow_p = cp.tile([P, DP1], F32, name=f"owt{p}")
                nc.scalar.dma_start(st_p[:], stt_d[p])
                nc.scalar.dma_start(x_p[:], xin_d[p])
                nc.scalar.dma_start(ow_p[:], ow_d[p])
                st_t.append(st_p)
                x_t.append(x_p)
                ow_t.append(ow_p)

            ar_in = dram.tile([D, W], F32, name="ar_in")
            ar_out = dram.tile([D, W], F32, name="ar_out")
            ag_in = dram.tile([S, D], F32, name="ag_in")
            ag_out = dram.tile([W, D], F32, name="ag_out")

            rm_t, og_g, ap_t, rg_t, sg_t, sn_t = [], [], [], [], [], []
            apf = []

            # ================= PHASE A: weights streaming ==================
            with (
                tc.tile_pool(name="iop", bufs=1) as iop,
                tc.tile_pool(name="wp", bufs=3) as wp,
                tc.tile_pool(name="pe1", bufs=1, space=bass.MemorySpace.PSUM) as pe1p,
                tc.tile_pool(name="pe2", bufs=1, space=bass.MemorySpace.PSUM) as pe2p,
                tc.tile_pool(name="pst", bufs=2, space=bass.MemorySpace.PSUM) as pst,
            ):
                sa_t, og_t, sgt = [], [], []
                for p in range(2):
                    sa_p = iop.tile([P, CT], F32, name=f"sa{p}")
                    og_p = iop.tile([P, CT], F32, name=f"og{p}")
                    nc.sync.dma_start(sa_p[:], sa_d[p])
                    nc.sync.dma_start(og_p[:], og_d[p])
                    sa_t.append(sa_p)
                    og_t.append(og_p)
                for jc in range(16):
                    sg_jc = cp.tile([P, D - 1], F32, name=f"sgt{jc}")
                    nc.scalar.dma_start(sg_jc[:], sgin_d[jc])
                    sgt.append(sg_jc)

                # diagonal stationary operands
                stdiag = []
                for p in range(2):
                    sd = cp.tile([P, D * D], F32, name=f"stdiag{p}")
                    nc.vector.memset(sd[:], 0.0)
                    nc.vector.tensor_copy(sd[:, bass.ds(0, D, D + 1)], st_t[p][:])
                    stdiag.append(sd)
                sgdiag = []
                for jc in range(16):
                    sd = cp.tile([P, (D - 1) * (D - 1)], F32, name=f"sgdiag{jc}")
                    nc.vector.memset(sd[:], 0.0)
                    nc.vector.tensor_copy(sd[:, bass.ds(0, D - 1, D)], sgt[jc][:])
                    sgdiag.append(sd)

                # og scatter + writeback (independent of collectives)
                for p in range(2):
                    nc.vector.tensor_copy(og_t[p][:, bass.ds(t, DP1, MOD)], ow_t[p][:])
                    nc.sync.dma_start(o_og[p], og_t[p][:])

                # einsum1: hidden partials
                pe1 = [pe1p.tile([D, 512], F32, name=f"pe1_{b}", tag=f"pe1_{b}")
                       for b in range(4)]
                for b in range(4):
                    for p in range(2):
                        wt = wp.tile([P, 4096], F32, name=f"wt{b}{p}", tag="slab")
                        nc.sync.dma_start(wt[:], slab_d[p, :, b * 4096 : (b + 1) * 4096])
                        for d in range(D):
                            nc.tensor.matmul(
                                pe1[b][:],
                                stdiag[p][:, D * d : D * (d + 1)],
                                wt[:, bass.ds(d, 512, D)],
                                start=(p == 0 and d == 0),
                                stop=(p == 1 and d == D - 1),
                            )

                # einsum2: contrib (local, complete)
                pe2 = pe2p.tile([D - 1, S], F32, name="pe2")
                for jc in range(16):
                    wtt = wp.tile([P, S * (D - 1)], F32, name=f"wtt{jc}", tag="slabT")
                    nc.sync.dma_start(wtt[:], slabT_d[jc])
                    for a in range(D - 1):
                        nc.tensor.matmul(
                            pe2[:],
                            sgdiag[jc][:, (D - 1) * a : (D - 1) * (a + 1)],
                            wtt[:, bass.ds(a, S, D - 1)],
                            start=(jc == 0 and a == 0),
                            stop=(jc == 15 and a == D - 2),
                        )

                # AllReduce of hidden^T partials
                ar_sb = iop.tile([D, W], F32, name="ar_sb")
                for b in range(4):
                    nc.vector.tensor_copy(ar_sb[:, b * 512 : (b + 1) * 512], pe1[b][:])
                nc.scalar.dma_start(ar_in[:], ar_sb[:])
                nc.gpsimd.collective_compute(
                    "AllReduce", ALU.add, replica_groups=rg,
                    ins=[ar_in.opt()], outs=[ar_out.opt()],
                )

                # gathers from sa/og tiles (old columns only)
                for p in range(2):
                    g1 = cp.tile([P, D - 1], F32, name=f"rm{p}")
                    _gather(nc, g1, sa_t[p], rm_off)
                    g2 = cp.tile([P, D - 1], F32, name=f"ogg{p}")
                    _gather(nc, g2, og_t[p], og_off)
                    g3 = cp.tile([P, D], F32, name=f"app{p}")
                    _gather(nc, g3, sa_t[p], ap_off)
                    g4 = cp.tile([P, D - 1], F32, name=f"rgg{p}")
                    _gather(nc, g4, sa_t[p], rg_off)
                    rm_t.append(g1)
                    og_g.append(g2)
                    ap_t.append(g3)
                    rg_t.append(g4)

                # sg = relu_m * contrib + og_old ; sg[:,7] = ow[:,8]
                cb = cp.tile([D - 1, S], F32, name="cb")
                nc.vector.tensor_copy(cb[:], pe2[:])
                for p in range(2):
                    pct = pst.tile([P, D - 1], F32, name=f"pct{p}", tag="pst")
                    nc.tensor.transpose(
                        pct[:], cb[:, p * P : (p + 1) * P], ident[0 : D - 1, 0 : D - 1]
                    )
                    sgA = cp.tile([P, D - 1], F32, name=f"sgA{p}")
                    nc.vector.scalar_tensor_tensor(
                        sgA[:], rm_t[p][:], 0.0, pct[:],
                        op0=ALU.is_gt, op1=ALU.mult,
                    )
                    sg_p = cp.tile([P, D], F32, name=f"sgp{p}")
                    nc.vector.tensor_add(sg_p[:, 0 : D - 1], sgA[:], og_g[p][:])
                    nc.vector.tensor_copy(sg_p[:, D - 1 : D], ow_t[p][:, D : DP1])
                    nc.scalar.dma_start(o_sg[p], sg_p[:])
                    sg_t.append(sg_p)

                # post-AR: hidden shard -> state_new -> sa scatter
                pid = nc.scalar.partition_id()
                hid_sb = cp.tile([D, S], F32, name="hid_sb")
                nc.scalar.dma_start(hid_sb[:], ar_out[:, bass.ds(pid * S, S)])
                for p in range(2):
                    ph = pst.tile([P, D], F32, name=f"ph{p}", tag="pst")
                    nc.tensor.transpose(
                        ph[:], hid_sb[:, p * P : (p + 1) * P], ident[0:D, 0:D]
                    )
                    sn = cp.tile([P, DP1], F32, name=f"sn{p}")
                    nc.vector.tensor_copy(sn[:, 0:1], x_t[p][:])
                    nc.vector.tensor_relu(sn[:, 1:DP1], ph[:])
                    nc.vector.tensor_copy(sa_t[p][:, bass.ds(t, DP1, MOD)], sn[:])
                    nc.sync.dma_start(o_sa[p], sa_t[p][:])
                    nc.scalar.dma_start(o_state[p], sn[:])
                    sn_t.append(sn)

                # col shards -> AllGather input
                for p in range(2):
                    col_p = cp.tile([P, D], F32, name=f"col{p}")
                    nc.vector.scalar_tensor_tensor(
                        col_p[:, 0 : D - 1], rg_t[p][:], 0.0, sg_t[p][:, 0 : D - 1],
                        op0=ALU.is_gt, op1=ALU.mult,
                    )
                    nc.vector.scalar_tensor_tensor(
                        col_p[:, D - 1 : D], sn_t[p][:, D : DP1], 0.0,
                        ow_t[p][:, D : DP1],
                        op0=ALU.is_gt, op1=ALU.mult,
                    )
                    nc.scalar.dma_start(ag_in[p * P : (p + 1) * P, :], col_p[:])
                nc.gpsimd.collective_compute(
                    "AllGather", ALU.bypass, replica_groups=rg,
                    ins=[ag_in.opt()], outs=[ag_out.opt()],
                )

            # ================= PHASE B: gradients ==========================
            with tc.tile_pool(name="pb", bufs=1) as pb, \
                 tc.tile_pool(name="stg", bufs=3) as stg:
                with tc.tile_pool(name="pst2", bufs=2, space=bass.MemorySpace.PSUM) as pst2:
                    # colT [8, 2048] -> flat [1, 16384]
                    agbig = pb.tile([P, P], F32, name="agbig")
                    agview = ag_out[:].rearrange("(c i) d -> i c d", i=P)
                    nc.scalar.dma_start(
                        agbig[:].rearrange("i (c d) -> i c d", c=16), agview
                    )
                    ct_sb = pb.tile([D, W], F32, name="ct_sb")
                    for c in range(16):
                        pct2 = pst2.tile([D, P], F32, name=f"pct2_{c}", tag="pst2")
                        nc.tensor.transpose(
                            pct2[:], agbig[:, c * D : (c + 1) * D], ident[:]
                        )
                        nc.vector.tensor_copy(ct_sb[:, c * P : (c + 1) * P], pct2[:])
                    ctflat = pb.tile([1, D * W], F32, name="ctflat")
                    nc.scalar.dma_start(ctflat[:], ct_sb[:])

                    # act_prev^T flat [1, 1024] per 128-row block
                    for p in range(2):
                        pap = pst2.tile([D, P], F32, name=f"pap{p}", tag="pst2")
                        nc.tensor.transpose(pap[:], ap_t[p][:], ident[:])
                        apT = pb.tile([D, P], F32, name=f"apT{p}")
                        nc.vector.tensor_copy(apT[:], pap[:])
                        apf_p = pb.tile([1, D * P], F32, name=f"apf{p}")
                        nc.scalar.dma_start(apf_p[:], apT[:])
                        apf.append(apf_p)

                # gradients: K=1 outer products, d-major PSUM, interleaved copy
                with tc.tile_pool(name="pg", bufs=2, space=bass.MemorySpace.PSUM) as pgp:
                    for ib in range(2):
                        for wc in range(8):
                            pgt = pgp.tile([P, W], F32, name=f"pg{ib}{wc}", tag="pg")
                            for d in range(D):
                                nc.tensor.matmul(
                                    pgt[:, d * S : (d + 1) * S],
                                    apf[ib][0:1, d * P : (d + 1) * P],
                                    ctflat[0:1, d * W + wc * S : d * W + (wc + 1) * S],
                                    start=True,
                                    stop=True,
                                )
                            stg_t = stg.tile([P, W], F32, name=f"stg{ib}{wc}", tag="stg")
                            src = pgt[:].rearrange("p (d w) -> p d w", d=D)
                            dst = stg_t[:].rearrange("p (w d) -> p d w", d=D)
                            if wc % 2 == 0:
                                nc.vector.tensor_copy(dst, src)
                            else:
                                nc.scalar.copy(dst, src)
                            nc.sync.dma_start(
                                o_gr[ib, :, wc * W : (wc + 1) * W], stg_t[:]
                            )

    nc.compile()
    return nc


_CACHE = {}


def _get(time):
    t = int(time)
    if t not in _CACHE:
        _CACHE[t] = _build(t)
    return _CACHE[t]


def _prep_in_maps(x, weights, output_weights, state, stored_activations,
                  stored_gradiets, output_gradient):
    x = np.ascontiguousarray(np.asarray(x, dtype=np.float32))
    weights = np.asarray(weights, dtype=np.float32)
    output_weights = np.ascontiguousarray(np.asarray(output_weights, dtype=np.float32))
    state = np.ascontiguousarray(np.asarray(state, dtype=np.float32))
    stored_activations = np.ascontiguousarray(np.asarray(stored_activations, dtype=np.float32))
    stored_gradiets = np.ascontiguousarray(np.asarray(stored_gradiets, dtype=np.float32))
    output_gradient = np.ascontiguousarray(np.asarray(output_gradient, dtype=np.float32))

    ident = np.eye(P, dtype=np.float32)
    sgin = np.ascontiguousarray(stored_gradiets[:, 1:D]).reshape(16, P, D - 1)
    in_maps = []
    for m in range(NCORES):
        sl = slice(m * S, (m + 1) * S)
        slab = np.ascontiguousarray(weights[sl]).reshape(2, P, W * D)
        slabT = np.ascontiguousarray(
            np.transpose(weights[sl, :, 1:D], (1, 0, 2))
        ).reshape(16, P, S * (D - 1))
        in_maps.append({
            "slab": slab,
            "slabT": slabT,
            "stt": np.ascontiguousarray(state[sl, 0:D]).reshape(2, P, D),
            "xin": np.ascontiguousarray(x[sl]).reshape(2, P, 1),
            "ow": output_weights[sl].reshape(2, P, DP1),
            "sgin": sgin,
            "ident": ident,
            "sa": stored_activations[sl].reshape(2, P, CT),
            "og": output_gradient[sl].reshape(2, P, CT),
        })
    return in_maps


def _assemble(results, output_weights):
    state = np.concatenate([r["o_state"].reshape(S, DP1) for r in results])
    sg = np.concatenate([r["o_sg"].reshape(S, D) for r in results])
    sa = np.concatenate([r["o_sa"].reshape(S, DP1, MOD) for r in results])
    og = np.concatenate([r["o_og"].reshape(S, DP1, MOD) for r in results])
    grads = np.concatenate([r["o_gr"].reshape(S, W, D) for r in results])
    output_weights = np.asarray(output_weights, dtype=np.float32)
    output = np.float32(np.sum(output_weights * state, dtype=np.float64))
    owg = state.copy()
    return (np.asarray(output, dtype=np.float32), state, sa, sg, og, grads, owg)


def _run(inputs, trace=False):
    time = int(inputs["time"])
    nc = _get(time)
    in_maps = _prep_in_maps(
        inputs["x"], inputs["weights"], inputs["output_weights"], inputs["state"],
        inputs["stored_activations"], inputs["stored_gradiets"],
        inputs["output_gradient"],
    )
    res = bass_utils.run_bass_kernel_spmd(
        nc, in_maps, core_ids=list(range(NCORES)), trace=trace
    )
    outs = _assemble(res.results, inputs["output_weights"])
    return outs, res


def kernel(**inputs):
    outs, _ = _run(inputs, trace=False)
    return outs


# revision 5
# speedup vs baseline: 1.2169x; 1.2169x over previous
"""Trainium2 Bass kernel for the recurrent-column step (nn_Column_23398981829106).

Sharding (8 NeuronCores, width in 256-row shards):
  - core m holds slab  = weights[256m:256m+256, :, :]      (natural layout)
            and slabT = weights[256m:256m+256, :, 1:8].T   (host pre-transposed, j-major)

Precision: the PE's native fp32 matmul runs in LOW_HIGH dual-pass mode at half
clock (~2.3x slower than bf16).  We do the same decomposition ourselves on the
host: W = Whi + Wlo (both bf16, same total bytes as f32), stream bf16 at full
rate and accumulate in f32 PSUM.  The stationary operands (state / stored
gradients / act_prev / col) are split on device the same way; all four
hi*hi, hi*lo, lo*hi, lo*lo products are kept, so the result matches f32 to
~1e-5 relative.

  - einsum1 hidden[w,d] = sum_i W[i,w,d] * state[i,d]: masked block-diagonal
    stationary operand (st hi/lo interleaved, M=16) accumulates all d into one
    [16, 512] PSUM per w-block over 2 passes (Whi, Wlo) -> partial
    hidden^T-by-halves [16, 2048] -> AllReduce -> each core slices + folds its
    own 256 columns (partition_id).
  - einsum2 contrib[i,a] = sum_j W[i,j,a+1] * sg[j,a+1]: local and complete on
    slabT (contracts j on PE partitions), [14, 256] PSUM, folded after
    transpose.
  - circular buffers sa / og: stream shard through SBUF, overwrite the 9
    columns at t=time, stream back out; gather columns are strided SBUF reads.
  - col = sg * relu_mask, split hi/lo -> AllGather (2048, 16) bf16 -> PE
    transposes -> ctstack [16, 2048] bf16 (rows: colT hi 0-7, lo 8-15).
  - gradients[i,w,d] = act_prev[i,d] * col[w,d]: K=4 bf16 matmuls
    (lhsT rows (ahi,alo,ahi,alo), rhs rows (chi,chi,clo,clo)) into
    [128, 2048] f32 PSUM (d-major), interleaved to (w,d)-major during the
    PSUM->SBUF copies (alternating DVE/ACT), then full-rate 1MB DMAs out.

Host side shards inputs, splits weights into bf16 hi/lo, sums the scalar
output and concatenates the per-core output shards.
"""

import sys

sys.path.insert(0, "/opt/trn_rl_repo")

import ml_dtypes
import numpy as np

import concourse.bass as bass
import concourse.bacc as bacc
import concourse.tile as tile
import concourse.mybir as mybir
from concourse import bass_utils

F32 = mybir.dt.float32
BF16 = mybir.dt.bfloat16
ALU = mybir.AluOpType
NPBF = ml_dtypes.bfloat16

P = 128          # partitions
W = 2048         # column width
D = 8            # depth
DP1 = D + 1
MOD = 30 * D + 2  # 242
S = 256          # shard rows per core
NCORES = 8
CT = DP1 * MOD   # 2178 flattened (c, t) free dim of sa/og shards


def _runs(offsets):
    """Group an increasing offset list into (dst_start, src_start, count, step)
    strided runs (one engine copy each)."""
    out = []
    i = 0
    n = len(offsets)
    while i < n:
        if i == n - 1:
            out.append((i, offsets[i], 1, 1))
            break
        d = offsets[i + 1] - offsets[i]
        j = i + 1
        while j + 1 < n and offsets[j + 1] - offsets[j] == d:
            j += 1
        if d <= 0:
            out.append((i, offsets[i], 1, 1))
            i += 1
        else:
            out.append((i, offsets[i], j - i + 1, d))
            i = j + 1
    return out


def _gather(nc, dst, src, offsets):
    for d0, s0, cnt, step in _runs(offsets):
        nc.vector.tensor_copy(dst[:, d0 : d0 + cnt], src[:, bass.ds(s0, cnt, step)])


def _split_hi_lo(nc, cp, src_f32, name, cols):
    """Device-side f32 -> (bf16 hi, bf16 lo) split of a [P, cols] tile."""
    hi = cp.tile([P, cols], BF16, name=f"{name}h")
    nc.vector.tensor_copy(hi[:], src_f32[:])
    lo_f = cp.tile([P, cols], F32, name=f"{name}lf")
    nc.vector.tensor_sub(lo_f[:], src_f32[:], hi[:])
    lo = cp.tile([P, cols], BF16, name=f"{name}l")
    nc.vector.tensor_copy(lo[:], lo_f[:])
    return hi, lo


def _build(time):
    t = int(time)
    tidx = [(t - 2 * D + 2 * (a + 1)) % MOD for a in range(D - 1)]  # a = 0..6
    tia = [(t - 2 * D + 2 * (aa + 1)) % MOD for aa in range(D)]     # aa = 0..7

    rm_off = [(a + 2) * MOD + (tidx[a] + 1) % MOD for a in range(D - 1)]
    og_off = [(a + 1) * MOD + tidx[a] for a in range(D - 1)]
    ap_off = [aa * MOD + (tia[aa] - 1) % MOD for aa in range(D)]
    rg_off = [(aa + 1) * MOD + tia[aa] for aa in range(D - 1)]
    for off in rm_off + og_off + ap_off + rg_off:
        assert off % MOD != t, (time, off)

    nc = bacc.Bacc("TRN2", target_bir_lowering=False, debug=False, num_devices=NCORES)

    slabh_d = nc.dram_tensor("slabh", [2, P, W * D], BF16, kind="ExternalInput")
    slabl_d = nc.dram_tensor("slabl", [2, P, W * D], BF16, kind="ExternalInput")
    slabTh_d = nc.dram_tensor("slabTh", [16, P, S * (D - 1)], BF16, kind="ExternalInput")
    slabTl_d = nc.dram_tensor("slabTl", [16, P, S * (D - 1)], BF16, kind="ExternalInput")
    stt_d = nc.dram_tensor("stt", [2, P, D], F32, kind="ExternalInput")
    xin_d = nc.dram_tensor("xin", [2, P, 1], F32, kind="ExternalInput")
    ow_d = nc.dram_tensor("ow", [2, P, DP1], F32, kind="ExternalInput")
    sgin_d = nc.dram_tensor("sgin", [16, P, D - 1], F32, kind="ExternalInput")
    ident_d = nc.dram_tensor("ident", [P, P], F32, kind="ExternalInput")
    sa_d = nc.dram_tensor("sa", [2, P, CT], F32, kind="ExternalInput")
    og_d = nc.dram_tensor("og", [2, P, CT], F32, kind="ExternalInput")

    o_state = nc.dram_tensor("o_state", [2, P, DP1], F32, kind="ExternalOutput")
    o_sg = nc.dram_tensor("o_sg", [2, P, D], F32, kind="ExternalOutput")
    o_sa = nc.dram_tensor("o_sa", [2, P, CT], F32, kind="ExternalOutput")
    o_og = nc.dram_tensor("o_og", [2, P, CT], F32, kind="ExternalOutput")
    o_gr = nc.dram_tensor("o_gr", [2, P, W * D], F32, kind="ExternalOutput")

    rg = [list(range(NCORES))]

    with tile.TileContext(nc) as tc:
        with tc.tile_pool(name="cp", bufs=1) as cp, \
             tc.tile_pool(name="dram", bufs=1, space="DRAM") as dram:
            # persistent small tiles
            ident = cp.tile([P, P], F32, name="ident")
            nc.scalar.dma_start(ident[:], ident_d[:])
            identb = cp.tile([P, P], BF16, name="identb")
            nc.vector.tensor_copy(identb[:], ident[:])
            st_t, x_t, ow_t = [], [], []
            for p in range(2):
                st_p = cp.tile([P, D], F32, name=f"st{p}")
                x_p = cp.tile([P, 1], F32, name=f"x{p}")
                ow_p = cp.tile([P, DP1], F32, name=f"owt{p}")
                nc.scalar.dma_start(st_p[:], stt_d[p])
                nc.scalar.dma_start(x_p[:], xin_d[p])
                nc.scalar.dma_start(ow_p[:], ow_d[p])
                st_t.append(st_p)
                x_t.append(x_p)
                ow_t.append(ow_p)

            ar_in = dram.tile([2 * D, W], F32, name="ar_in")
            ar_out = dram.tile([2 * D, W], F32, name="ar_out")
            ag_in = dram.tile([S, 2 * D], BF16, name="ag_in")
            ag_out = dram.tile([W, 2 * D], BF16, name="ag_out")

            rm_t, og_g, ap_t, rg_t, sg_t, sn_t = [], [], [], [], [], []
            apstack = []

            # ================= PHASE A: weights streaming ==================
            with (
                tc.tile_pool(name="iop", bufs=1) as iop,
                tc.tile_pool(name="wp", bufs=3) as wp,
                tc.tile_pool(name="pe1", bufs=1, space=bass.MemorySpace.PSUM) as pe1p,
                tc.tile_pool(name="pe2", bufs=1, space=bass.MemorySpace.PSUM) as pe2p,
                tc.tile_pool(name="pst", bufs=2, space=bass.MemorySpace.PSUM) as pst,
            ):
                # stationary operands: state split into bf16 hi/lo, interleaved
                # block-diagonal [128, 128]: col 16d + 2d -> sthi[:,d],
                # col 16d + 2d + 1 -> stlo[:,d]
                stdiag = []
                for p in range(2):
                    hi, lo = _split_hi_lo(nc, cp, st_t[p], f"st{p}", D)
                    sd = cp.tile([P, 16 * D], BF16, name=f"stdiag{p}")
                    nc.vector.memset(sd[:], 0.0)
                    nc.vector.tensor_copy(sd[:, bass.ds(0, D, 18)], hi[:])
                    nc.vector.tensor_copy(sd[:, bass.ds(1, D, 18)], lo[:])
                    stdiag.append(sd)

                sgt, sgdiag = [], []
                for jc in range(16):
                    sg_jc = cp.tile([P, D - 1], F32, name=f"sgt{jc}")
                    nc.gpsimd.dma_start(sg_jc[:], sgin_d[jc])
                    sgt.append(sg_jc)
                    hi, lo = _split_hi_lo(nc, cp, sg_jc, f"sg{jc}", D - 1)
                    sd = cp.tile([P, 14 * (D - 1)], BF16, name=f"sgdiag{jc}")
                    nc.vector.memset(sd[:], 0.0)
                    nc.vector.tensor_copy(sd[:, bass.ds(0, D - 1, 16)], hi[:])
                    nc.vector.tensor_copy(sd[:, bass.ds(1, D - 1, 16)], lo[:])
                    sgdiag.append(sd)

                # einsum1: hidden partials (hi/lo rows interleaved, M=16)
                pe1 = [pe1p.tile([2 * D, 512], F32, name=f"pe1_{b}", tag=f"pe1_{b}")
                       for b in range(4)]
                for b in range(4):
                    for p in range(2):
                        wth = wp.tile([P, 4096], BF16, name=f"wth{b}{p}", tag="slabh")
                        wtl = wp.tile([P, 4096], BF16, name=f"wtl{b}{p}", tag="slabl")
                        nc.sync.dma_start(wth[:], slabh_d[p, :, b * 4096 : (b + 1) * 4096])
                        nc.sync.dma_start(wtl[:], slabl_d[p, :, b * 4096 : (b + 1) * 4096])
                        for d in range(D):
                            nc.tensor.matmul(
                                pe1[b][:],
                                stdiag[p][:, 16 * d : 16 * (d + 1)],
                                wth[:, bass.ds(d, 512, D)],
                                start=(p == 0 and d == 0),
                                stop=False,
                            )
                            nc.tensor.matmul(
                                pe1[b][:],
                                stdiag[p][:, 16 * d : 16 * (d + 1)],
                                wtl[:, bass.ds(d, 512, D)],
                                start=False,
                                stop=(p == 1 and d == D - 1),
                            )

                # einsum2: contrib, local (hi/lo rows interleaved, M=14)
                pe2 = pe2p.tile([14, S], F32, name="pe2")
                for jc in range(16):
                    wtth = wp.tile([P, S * (D - 1)], BF16, name=f"wtth{jc}", tag="slabTh")
                    wttl = wp.tile([P, S * (D - 1)], BF16, name=f"wttl{jc}", tag="slabTl")
                    nc.sync.dma_start(wtth[:], slabTh_d[jc])
                    nc.sync.dma_start(wttl[:], slabTl_d[jc])
                    for a in range(D - 1):
                        nc.tensor.matmul(
                            pe2[:],
                            sgdiag[jc][:, 14 * a : 14 * (a + 1)],
                            wtth[:, bass.ds(a, S, D - 1)],
                            start=(jc == 0 and a == 0),
                            stop=False,
                        )
                        nc.tensor.matmul(
                            pe2[:],
                            sgdiag[jc][:, 14 * a : 14 * (a + 1)],
                            wttl[:, bass.ds(a, S, D - 1)],
                            start=False,
                            stop=(jc == 15 and a == D - 2),
                        )

                # AllReduce of hidden^T partials (rows 2d = hi, 2d+1 = lo)
                ar_sb = iop.tile([2 * D, W], F32, name="ar_sb")
                for b in range(4):
                    nc.vector.tensor_copy(ar_sb[:, b * 512 : (b + 1) * 512], pe1[b][:])
                nc.scalar.dma_start(ar_in[:], ar_sb[:])
                nc.gpsimd.collective_compute(
                    "AllReduce", ALU.add, replica_groups=rg,
                    ins=[ar_in.opt()], outs=[ar_out.opt()],
                )

                # sa/og circular buffers
                sa_t, og_t = [], []
                for p in range(2):
                    sa_p = iop.tile([P, CT], F32, name=f"sa{p}")
                    og_p = iop.tile([P, CT], F32, name=f"og{p}")
                    nc.sync.dma_start(sa_p[:], sa_d[p])
                    nc.sync.dma_start(og_p[:], og_d[p])
                    sa_t.append(sa_p)
                    og_t.append(og_p)
                    nc.vector.tensor_copy(og_p[:, bass.ds(t, DP1, MOD)], ow_t[p][:])
                    nc.sync.dma_start(o_og[p], og_p[:])

                # gathers from sa/og tiles (old columns only)
                for p in range(2):
                    g1 = cp.tile([P, D - 1], F32, name=f"rm{p}")
                    _gather(nc, g1, sa_t[p], rm_off)
                    g2 = cp.tile([P, D - 1], F32, name=f"ogg{p}")
                    _gather(nc, g2, og_t[p], og_off)
                    g3 = cp.tile([P, D], F32, name=f"app{p}")
                    _gather(nc, g3, sa_t[p], ap_off)
                    g4 = cp.tile([P, D - 1], F32, name=f"rgg{p}")
                    _gather(nc, g4, sa_t[p], rg_off)
                    rm_t.append(g1)
                    og_g.append(g2)
                    ap_t.append(g3)
                    rg_t.append(g4)

                # act_prev^T stack [16, 128] bf16 per block (rows: hi 0-7, lo 8-15)
                for p in range(2):
                    hi, lo = _split_hi_lo(nc, cp, ap_t[p], f"ap{p}", D)
                    both = cp.tile([P, 2 * D], BF16, name=f"apb{p}")
                    nc.vector.tensor_copy(both[:, bass.ds(0, D, 2)], hi[:])
                    nc.vector.tensor_copy(both[:, bass.ds(1, D, 2)], lo[:])
                    pap = pst.tile([2 * D, P], BF16, name=f"pap{p}", tag="pst")
                    nc.tensor.transpose(pap[:], both[:], identb[:])
                    aps = cp.tile([2 * D, P], BF16, name=f"apstack{p}")
                    nc.vector.tensor_copy(aps[:], pap[:])
                    apstack.append(aps)

                # sg = relu_m * contrib + og_old ; sg[:,7] = ow[:,8]
                cb = cp.tile([14, S], F32, name="cb")
                nc.vector.tensor_copy(cb[:], pe2[:])
                for p in range(2):
                    pct = pst.tile([P, 14], F32, name=f"pct{p}", tag="pst")
                    nc.tensor.transpose(
                        pct[:], cb[:, p * P : (p + 1) * P], ident[0:14, 0:14]
                    )
                    ctr14 = cp.tile([P, 14], F32, name=f"ctr14_{p}")
                    nc.vector.tensor_copy(ctr14[:], pct[:])
                    ctr = cp.tile([P, D - 1], F32, name=f"ctr{p}")
                    nc.vector.tensor_add(
                        ctr[:], ctr14[:, bass.ds(0, D - 1, 2)],
                        ctr14[:, bass.ds(1, D - 1, 2)],
                    )
                    sgA = cp.tile([P, D - 1], F32, name=f"sgA{p}")
                    nc.vector.scalar_tensor_tensor(
                        sgA[:], rm_t[p][:], 0.0, ctr[:],
                        op0=ALU.is_gt, op1=ALU.mult,
                    )
                    sg_p = cp.tile([P, D], F32, name=f"sgp{p}")
                    nc.vector.tensor_add(sg_p[:, 0 : D - 1], sgA[:], og_g[p][:])
                    nc.vector.tensor_copy(sg_p[:, D - 1 : D], ow_t[p][:, D : DP1])
                    nc.scalar.dma_start(o_sg[p], sg_p[:])
                    sg_t.append(sg_p)

                # post-AR: fold hi/lo rows of own shard -> state_new -> sa scatter
                pid = nc.scalar.partition_id()
                base = pid * S
                hidh = cp.tile([D, S], F32, name="hidh")
                hidl = cp.tile([D, S], F32, name="hidl")
                nc.scalar.dma_start(hidh[:], ar_out[bass.ds(0, D, 2), bass.ds(base, S)])
                nc.scalar.dma_start(hidl[:], ar_out[bass.ds(1, D, 2), bass.ds(base, S)])
                hid_sb = cp.tile([D, S], F32, name="hid_sb")
                nc.vector.tensor_add(hid_sb[:], hidh[:], hidl[:])
                for p in range(2):
                    ph = pst.tile([P, D], F32, name=f"ph{p}", tag="pst")
                    nc.tensor.transpose(
                        ph[:], hid_sb[:, p * P : (p + 1) * P], ident[0:D, 0:D]
                    )
                    sn = cp.tile([P, DP1], F32, name=f"sn{p}")
                    nc.vector.tensor_copy(sn[:, 0:1], x_t[p][:])
                    nc.vector.tensor_relu(sn[:, 1:DP1], ph[:])
                    nc.vector.tensor_copy(sa_t[p][:, bass.ds(t, DP1, MOD)], sn[:])
                    nc.sync.dma_start(o_sa[p], sa_t[p][:])
                    nc.scalar.dma_start(o_state[p], sn[:])
                    sn_t.append(sn)

                # col shards (hi | lo bf16) -> AllGather
                for p in range(2):
                    col_p = cp.tile([P, D], F32, name=f"col{p}")
                    nc.vector.scalar_tensor_tensor(
                        col_p[:, 0 : D - 1], rg_t[p][:], 0.0, sg_t[p][:, 0 : D - 1],
                        op0=ALU.is_gt, op1=ALU.mult,
                    )
                    nc.vector.scalar_tensor_tensor(
                        col_p[:, D - 1 : D], sn_t[p][:, D : DP1], 0.0,
                        ow_t[p][:, D : DP1],
                        op0=ALU.is_gt, op1=ALU.mult,
                    )
                    chi, clo = _split_hi_lo(nc, cp, col_p, f"col{p}", D)
                    colb = cp.tile([P, 2 * D], BF16, name=f"colb{p}")
                    nc.vector.tensor_copy(colb[:, bass.ds(0, D, 2)], chi[:])
                    nc.vector.tensor_copy(colb[:, bass.ds(1, D, 2)], clo[:])
                    nc.scalar.dma_start(ag_in[p * P : (p + 1) * P, :], colb[:])
                nc.gpsimd.collective_compute(
                    "AllGather", ALU.bypass, replica_groups=rg,
                    ins=[ag_in.opt()], outs=[ag_out.opt()],
                )

            # ================= PHASE B: gradients ==========================
            with tc.tile_pool(name="pb", bufs=1) as pb, \
                 tc.tile_pool(name="stg", bufs=3) as stg:
                with tc.tile_pool(name="pst2", bufs=2, space=bass.MemorySpace.PSUM) as pst2:
                    # colT stack [16, 2048] bf16 (rows: hi d=0..7, lo d=0..7)
                    agbig = pb.tile([P, 16 * 2 * D], BF16, name="agbig")
                    agview = ag_out[:].rearrange("(c i) hd -> i c hd", i=P)
                    nc.scalar.dma_start(
                        agbig[:].rearrange("i (c hd) -> i c hd", c=16), agview
                    )
                    ctstack = pb.tile([2 * D, W], BF16, name="ctstack")
                    for c in range(16):
                        pct2 = pst2.tile([2 * D, P], BF16, name=f"pct2_{c}", tag="pst2")
                        nc.tensor.transpose(
                            pct2[:], agbig[:, c * 16 : (c + 1) * 16], identb[:]
                        )
                        nc.vector.tensor_copy(ctstack[:, c * P : (c + 1) * P], pct2[:])

                # gradients: K=4 bf16 outer products
                # lhsT rows (ahi_d, alo_d, ahi_d, alo_d); rhs rows (chi_d, chi_d, clo_d, clo_d)
                lh4 = [[None] * D for _ in range(2)]
                rhs4 = [None] * D
                for d in range(D):
                    r4 = pb.tile([4, W], BF16, name=f"rhs4_{d}")
                    nc.gpsimd.dma_start(r4[0:1, :], ctstack[2 * d : 2 * d + 1, :])
                    nc.gpsimd.dma_start(r4[1:2, :], ctstack[2 * d : 2 * d + 1, :])
                    nc.gpsimd.dma_start(r4[2:3, :], ctstack[2 * d + 1 : 2 * d + 2, :])
                    nc.gpsimd.dma_start(r4[3:4, :], ctstack[2 * d + 1 : 2 * d + 2, :])
                    rhs4[d] = r4
                    for ib in range(2):
                        l4 = pb.tile([4, P], BF16, name=f"lh4_{ib}_{d}")
                        nc.gpsimd.dma_start(l4[0:2, :], apstack[ib][2 * d : 2 * d + 2, :])
                        nc.gpsimd.dma_start(l4[2:4, :], apstack[ib][2 * d : 2 * d + 2, :])
                        lh4[ib][d] = l4

                with tc.tile_pool(name="pg", bufs=2, space=bass.MemorySpace.PSUM) as pgp:
                    for ib in range(2):
                        for wc in range(8):
                            pgt = pgp.tile([P, W], F32, name=f"pg{ib}{wc}", tag="pg")
                            for d in range(D):
                                nc.tensor.matmul(
                                    pgt[:, d * S : (d + 1) * S],
                                    lh4[ib][d][:],
                                    rhs4[d][:, wc * S : (wc + 1) * S],
                                    start=True,
                                    stop=True,
                                )
                            stg_t = stg.tile([P, W], F32, name=f"stg{ib}{wc}", tag="stg")
                            src = pgt[:].rearrange("p (d w) -> p d w", d=D)
                            dst = stg_t[:].rearrange("p (w d) -> p d w", d=D)
                            if wc % 2 == 0:
                                nc.vector.tensor_copy(dst, src)
                            else:
                                nc.scalar.copy(dst, src)
                            nc.sync.dma_start(
                                o_gr[ib, :, wc * W : (wc + 1) * W], stg_t[:]
                            )

    nc.compile()
    return nc


_CACHE = {}


def _get(time):
    t = int(time)
    if t not in _CACHE:
        _CACHE[t] = _build(t)
    return _CACHE[t]


def _prep_in_maps(x, weights, output_weights, state, stored_activations,
                  stored_gradiets, output_gradient):
    x = np.ascontiguousarray(np.asarray(x, dtype=np.float32))
    weights = np.asarray(weights, dtype=np.float32)
    output_weights = np.ascontiguousarray(np.asarray(output_weights, dtype=np.float32))
    state = np.ascontiguousarray(np.asarray(state, dtype=np.float32))
    stored_activations = np.ascontiguousarray(np.asarray(stored_activations, dtype=np.float32))
    stored_gradiets = np.ascontiguousarray(np.asarray(stored_gradiets, dtype=np.float32))
    output_gradient = np.ascontiguousarray(np.asarray(output_gradient, dtype=np.float32))

    ident = np.eye(P, dtype=np.float32)
    sgin = np.ascontiguousarray(stored_gradiets[:, 1:D]).reshape(16, P, D - 1)

    whi = weights.astype(NPBF)
    wlo = (weights - whi.astype(np.float32)).astype(NPBF)

    in_maps = []
    for m in range(NCORES):
        sl = slice(m * S, (m + 1) * S)
        slabh = np.ascontiguousarray(whi[sl]).reshape(2, P, W * D)
        slabl = np.ascontiguousarray(wlo[sl]).reshape(2, P, W * D)
        slabTh = np.ascontiguousarray(
            np.transpose(whi[sl, :, 1:D], (1, 0, 2))
        ).reshape(16, P, S * (D - 1))
        slabTl = np.ascontiguousarray(
            np.transpose(wlo[sl, :, 1:D], (1, 0, 2))
        ).reshape(16, P, S * (D - 1))
        in_maps.append({
            "slabh": slabh,
            "slabl": slabl,
            "slabTh": slabTh,
            "slabTl": slabTl,
            "stt": np.ascontiguousarray(state[sl, 0:D]).reshape(2, P, D),
            "xin": np.ascontiguousarray(x[sl]).reshape(2, P, 1),
            "ow": output_weights[sl].reshape(2, P, DP1),
            "sgin": sgin,
            "ident": ident,
            "sa": stored_activations[sl].reshape(2, P, CT),
            "og": output_gradient[sl].reshape(2, P, CT),
        })
    return in_maps


def _assemble(results, output_weights):
    state = np.concatenate([r["o_state"].reshape(S, DP1) for r in results])
    sg = np.concatenate([r["o_sg"].reshape(S, D) for r in results])
    sa = np.concatenate([r["o_sa"].reshape(S, DP1, MOD) for r in results])
    og = np.concatenate([r["o_og"].reshape(S, DP1, MOD) for r in results])
    grads = np.concatenate([r["o_gr"].reshape(S, W, D) for r in results])
    output_weights = np.asarray(output_weights, dtype=np.float32)
    output = np.float32(np.sum(output_weights * state, dtype=np.float64))
    owg = state.copy()
    return (np.asarray(output, dtype=np.float32), state, sa, sg, og, grads, owg)


def _run(inputs, trace=False):
    time = int(inputs["time"])
    nc = _get(time)
    in_maps = _prep_in_maps(
        inputs["x"], inputs["weights"], inputs["output_weights"], inputs["state"],
        inputs["stored_activations"], inputs["stored_gradiets"],
        inputs["output_gradient"],
    )
    res = bass_utils.run_bass_kernel_spmd(
        nc, in_maps, core_ids=list(range(NCORES)), trace=trace
    )
    outs = _assemble(res.results, inputs["output_weights"])
    return outs, res


def kernel(**inputs):
    outs, _ = _run(inputs, trace=False)
    return outs


# revision 6
# speedup vs baseline: 1.5018x; 1.2341x over previous
"""Trainium2 Bass kernel for the recurrent-column step (nn_Column_23398981829106).

Sharding (8 NeuronCores, width in 256-row shards):
  - core m holds slab  = weights[256m:256m+256, :, :]      (natural layout)
            and slabT = weights[256m:256m+256, :, 1:8].T   (host pre-transposed, j-major)

Precision: the PE's native fp32 matmul runs in LOW_HIGH dual-pass mode at half
clock (~2.3x slower than bf16).  We do the same decomposition ourselves on the
host: W = Whi + Wlo (both bf16, same total bytes as f32), stream bf16 at full
rate and accumulate in f32 PSUM.  The stationary operands (state / stored
gradients / act_prev / col) are split on device the same way; all four
hi*hi, hi*lo, lo*hi, lo*lo products are kept, so the result matches f32 to
~1e-5 relative.

  - einsum1 hidden[w,d] = sum_i W[i,w,d] * state[i,d]: masked block-diagonal
    stationary operand (st hi/lo interleaved, M=16) accumulates all d into one
    [16, 512] PSUM per w-block over 2 passes (Whi, Wlo) -> partial
    hidden^T-by-halves [16, 2048] -> AllReduce -> each core slices + folds its
    own 256 columns (partition_id).
  - einsum2 contrib[i,a] = sum_j W[i,j,a+1] * sg[j,a+1]: local and complete on
    slabT (contracts j on PE partitions), [14, 256] PSUM, folded after
    transpose.
  - circular buffers sa / og: stream shard through SBUF, overwrite the 9
    columns at t=time, stream back out; gather columns are strided SBUF reads.
  - col = sg * relu_mask, split hi/lo -> AllGather (2048, 16) bf16 -> PE
    transposes -> ctstack [16, 2048] bf16 (rows: colT hi 0-7, lo 8-15).
  - gradients[i,w,d] = act_prev[i,d] * col[w,d]: K=4 bf16 matmuls
    (lhsT rows (ahi,alo,ahi,alo), rhs rows (chi,chi,clo,clo)) into
    [128, 2048] f32 PSUM (d-major), interleaved to (w,d)-major during the
    PSUM->SBUF copies (alternating DVE/ACT), then full-rate 1MB DMAs out.

Host side shards inputs, splits weights into bf16 hi/lo, sums the scalar
output and concatenates the per-core output shards.
"""

import sys

sys.path.insert(0, "/opt/trn_rl_repo")

import ml_dtypes
import numpy as np

import concourse.bass as bass
import concourse.bacc as bacc
import concourse.tile as tile
import concourse.mybir as mybir
from concourse import bass_utils

F32 = mybir.dt.float32
BF16 = mybir.dt.bfloat16
ALU = mybir.AluOpType
NPBF = ml_dtypes.bfloat16

P = 128          # partitions
W = 2048         # column width
D = 8            # depth
DP1 = D + 1
MOD = 30 * D + 2  # 242
S = 256          # shard rows per core
NCORES = 8
CT = DP1 * MOD   # 2178 flattened (c, t) free dim of sa/og shards


def _runs(offsets):
    """Group an increasing offset list into (dst_start, src_start, count, step)
    strided runs (one engine copy each)."""
    out = []
    i = 0
    n = len(offsets)
    while i < n:
        if i == n - 1:
            out.append((i, offsets[i], 1, 1))
            break
        d = offsets[i + 1] - offsets[i]
        j = i + 1
        while j + 1 < n and offsets[j + 1] - offsets[j] == d:
            j += 1
        if d <= 0:
            out.append((i, offsets[i], 1, 1))
            i += 1
        else:
            out.append((i, offsets[i], j - i + 1, d))
            i = j + 1
    return out


def _gather(nc, dst, src, offsets):
    for d0, s0, cnt, step in _runs(offsets):
        nc.vector.tensor_copy(dst[:, d0 : d0 + cnt], src[:, bass.ds(s0, cnt, step)])


def _split_hi_lo(nc, cp, src_f32, name, cols):
    """Device-side f32 -> (bf16 hi, bf16 lo) split of a [P, cols] tile."""
    hi = cp.tile([P, cols], BF16, name=f"{name}h")
    nc.vector.tensor_copy(hi[:], src_f32[:])
    lo_f = cp.tile([P, cols], F32, name=f"{name}lf")
    nc.vector.tensor_sub(lo_f[:], src_f32[:], hi[:])
    lo = cp.tile([P, cols], BF16, name=f"{name}l")
    nc.vector.tensor_copy(lo[:], lo_f[:])
    return hi, lo


def _build(time):
    t = int(time)
    tidx = [(t - 2 * D + 2 * (a + 1)) % MOD for a in range(D - 1)]  # a = 0..6
    tia = [(t - 2 * D + 2 * (aa + 1)) % MOD for aa in range(D)]     # aa = 0..7

    rm_off = [(a + 2) * MOD + (tidx[a] + 1) % MOD for a in range(D - 1)]
    og_off = [(a + 1) * MOD + tidx[a] for a in range(D - 1)]
    ap_off = [aa * MOD + (tia[aa] - 1) % MOD for aa in range(D)]
    rg_off = [(aa + 1) * MOD + tia[aa] for aa in range(D - 1)]
    for off in rm_off + og_off + ap_off + rg_off:
        assert off % MOD != t, (time, off)

    nc = bacc.Bacc("TRN2", target_bir_lowering=False, debug=False, num_devices=NCORES)

    slabh_d = nc.dram_tensor("slabh", [2, P, W * D], BF16, kind="ExternalInput")
    slabl_d = nc.dram_tensor("slabl", [2, P, W * D], BF16, kind="ExternalInput")
    slabTh_d = nc.dram_tensor("slabTh", [16, P, S * (D - 1)], BF16, kind="ExternalInput")
    slabTl_d = nc.dram_tensor("slabTl", [16, P, S * (D - 1)], BF16, kind="ExternalInput")
    stt_d = nc.dram_tensor("stt", [2, P, D], F32, kind="ExternalInput")
    xin_d = nc.dram_tensor("xin", [2, P, 1], F32, kind="ExternalInput")
    ow_d = nc.dram_tensor("ow", [2, P, DP1], F32, kind="ExternalInput")
    sgin_d = nc.dram_tensor("sgin", [16, P, D - 1], F32, kind="ExternalInput")
    ident_d = nc.dram_tensor("ident", [P, P], F32, kind="ExternalInput")
    sa_d = nc.dram_tensor("sa", [2, P, CT], F32, kind="ExternalInput")
    og_d = nc.dram_tensor("og", [2, P, CT], F32, kind="ExternalInput")

    o_state = nc.dram_tensor("o_state", [2, P, DP1], F32, kind="ExternalOutput")
    o_sg = nc.dram_tensor("o_sg", [2, P, D], F32, kind="ExternalOutput")
    o_sa = nc.dram_tensor("o_sa", [2, P, CT], F32, kind="ExternalOutput")
    o_og = nc.dram_tensor("o_og", [2, P, CT], F32, kind="ExternalOutput")
    o_gr = nc.dram_tensor("o_gr", [2, P, W * D], F32, kind="ExternalOutput")

    rg = [list(range(NCORES))]

    with tile.TileContext(nc) as tc:
        with tc.tile_pool(name="cp", bufs=1) as cp, \
             tc.tile_pool(name="dram", bufs=1, space="DRAM") as dram:
            # persistent small tiles
            ident = cp.tile([P, P], F32, name="ident")
            nc.scalar.dma_start(ident[:], ident_d[:])
            identb = cp.tile([P, P], BF16, name="identb")
            nc.vector.tensor_copy(identb[:], ident[:])
            st_t, x_t, ow_t = [], [], []
            for p in range(2):
                st_p = cp.tile([P, D], F32, name=f"st{p}")
                x_p = cp.tile([P, 1], F32, name=f"x{p}")
                ow_p = cp.tile([P, DP1], F32, name=f"owt{p}")
                nc.scalar.dma_start(st_p[:], stt_d[p])
                nc.scalar.dma_start(x_p[:], xin_d[p])
                nc.scalar.dma_start(ow_p[:], ow_d[p])
                st_t.append(st_p)
                x_t.append(x_p)
                ow_t.append(ow_p)

            warm_in = dram.tile([1, 16], F32, name="warm_in")
            warm_out = dram.tile([1, 16], F32, name="warm_out")
            warm_sb = cp.tile([1, 16], F32, name="warm_sb")
            nc.vector.memset(warm_sb[:], 0.0)
            nc.gpsimd.dma_start(warm_in[:], warm_sb[:])
            nc.gpsimd.collective_compute(
                "AllReduce", ALU.add, replica_groups=rg,
                ins=[warm_in.opt()], outs=[warm_out.opt()],
            )
            ar_in = dram.tile([NCORES, 2 * D, S], F32, name="ar_in")
            rs_out = dram.tile([2 * D, S], F32, name="rs_out")
            ag_in = dram.tile([S, 2 * D], BF16, name="ag_in")
            ag_out = dram.tile([W, 2 * D], BF16, name="ag_out")

            rm_t, og_g, ap_t, rg_t, sg_t, sn_t = [], [], [], [], [], []
            apstack = []

            # ================= PHASE A: weights streaming ==================
            with (
                tc.tile_pool(name="iop", bufs=1) as iop,
                tc.tile_pool(name="wp", bufs=3) as wp,
                tc.tile_pool(name="pe1", bufs=1, space=bass.MemorySpace.PSUM) as pe1p,
                tc.tile_pool(name="pe2", bufs=1, space=bass.MemorySpace.PSUM) as pe2p,
                tc.tile_pool(name="pst", bufs=2, space=bass.MemorySpace.PSUM) as pst,
            ):
                # stationary operands: state split into bf16 hi/lo, interleaved
                # block-diagonal [128, 128]: col 16d + 2d -> sthi[:,d],
                # col 16d + 2d + 1 -> stlo[:,d]
                stdiag = []
                for p in range(2):
                    hi, lo = _split_hi_lo(nc, cp, st_t[p], f"st{p}", D)
                    sd = cp.tile([P, 16 * D], BF16, name=f"stdiag{p}")
                    nc.vector.memset(sd[:], 0.0)
                    nc.vector.tensor_copy(sd[:, bass.ds(0, D, 18)], hi[:])
                    nc.vector.tensor_copy(sd[:, bass.ds(1, D, 18)], lo[:])
                    stdiag.append(sd)

                sgt, sgdiag = [], []
                for jc in range(16):
                    sg_jc = cp.tile([P, D - 1], F32, name=f"sgt{jc}")
                    nc.gpsimd.dma_start(sg_jc[:], sgin_d[jc])
                    sgt.append(sg_jc)
                    hi, lo = _split_hi_lo(nc, cp, sg_jc, f"sg{jc}", D - 1)
                    sd = cp.tile([P, 14 * (D - 1)], BF16, name=f"sgdiag{jc}")
                    nc.vector.memset(sd[:], 0.0)
                    nc.vector.tensor_copy(sd[:, bass.ds(0, D - 1, 16)], hi[:])
                    nc.vector.tensor_copy(sd[:, bass.ds(1, D - 1, 16)], lo[:])
                    sgdiag.append(sd)

                # einsum1: hidden partials (hi/lo rows interleaved, M=16)
                pe1 = [pe1p.tile([2 * D, 512], F32, name=f"pe1_{b}", tag=f"pe1_{b}")
                       for b in range(4)]
                for b in range(4):
                    for p in range(2):
                        wth = wp.tile([P, 4096], BF16, name=f"wth{b}{p}", tag="slabh")
                        wtl = wp.tile([P, 4096], BF16, name=f"wtl{b}{p}", tag="slabl")
                        src_h = slabh_d[p].rearrange("i (d w) -> i d w", d=D)[:, :, b * 512 : (b + 1) * 512]
                        src_l = slabl_d[p].rearrange("i (d w) -> i d w", d=D)[:, :, b * 512 : (b + 1) * 512]
                        nc.sync.dma_start(wth[:].rearrange("i (d w) -> i d w", d=D), src_h)
                        nc.sync.dma_start(wtl[:].rearrange("i (d w) -> i d w", d=D), src_l)
                        for d in range(D):
                            nc.tensor.matmul(
                                pe1[b][:],
                                stdiag[p][:, 16 * d : 16 * (d + 1)],
                                wth[:, d * 512 : (d + 1) * 512],
                                start=(p == 0 and d == 0),
                                stop=False,
                            )
                            nc.tensor.matmul(
                                pe1[b][:],
                                stdiag[p][:, 16 * d : 16 * (d + 1)],
                                wtl[:, d * 512 : (d + 1) * 512],
                                start=False,
                                stop=(p == 1 and d == D - 1),
                            )

                # einsum2: contrib, local (hi/lo rows interleaved, M=14)
                pe2 = pe2p.tile([14, S], F32, name="pe2")
                for jc in range(16):
                    wtth = wp.tile([P, S * (D - 1)], BF16, name=f"wtth{jc}", tag="slabTh")
                    wttl = wp.tile([P, S * (D - 1)], BF16, name=f"wttl{jc}", tag="slabTl")
                    nc.sync.dma_start(wtth[:], slabTh_d[jc])
                    nc.sync.dma_start(wttl[:], slabTl_d[jc])
                    for a in range(D - 1):
                        nc.tensor.matmul(
                            pe2[:],
                            sgdiag[jc][:, 14 * a : 14 * (a + 1)],
                            wtth[:, a * S : (a + 1) * S],
                            start=(jc == 0 and a == 0),
                            stop=False,
                        )
                        nc.tensor.matmul(
                            pe2[:],
                            sgdiag[jc][:, 14 * a : 14 * (a + 1)],
                            wttl[:, a * S : (a + 1) * S],
                            start=False,
                            stop=(jc == 15 and a == D - 2),
                        )

                # AllReduce of hidden^T partials (rows 2d = hi, 2d+1 = lo)
                ar_sb = iop.tile([2 * D, W], F32, name="ar_sb")
                for b in range(4):
                    nc.vector.tensor_copy(ar_sb[:, b * 512 : (b + 1) * 512], pe1[b][:])
                nc.scalar.dma_start(
                    ar_in[:].rearrange("s r i -> r s i"),
                    ar_sb[:].rearrange("r (s i) -> r s i", s=NCORES),
                )
                nc.gpsimd.collective_compute(
                    "ReduceScatter", ALU.add, replica_groups=rg,
                    ins=[ar_in.opt()], outs=[rs_out.opt()],
                )

                # sa/og circular buffers
                sa_t, og_t = [], []
                for p in range(2):
                    sa_p = iop.tile([P, CT], F32, name=f"sa{p}")
                    og_p = iop.tile([P, CT], F32, name=f"og{p}")
                    nc.sync.dma_start(sa_p[:], sa_d[p])
                    nc.sync.dma_start(og_p[:], og_d[p])
                    sa_t.append(sa_p)
                    og_t.append(og_p)
                    nc.vector.tensor_copy(og_p[:, bass.ds(t, DP1, MOD)], ow_t[p][:])
                    nc.sync.dma_start(o_og[p], og_p[:])

                # gathers from sa/og tiles (old columns only)
                for p in range(2):
                    g1 = cp.tile([P, D - 1], F32, name=f"rm{p}")
                    _gather(nc, g1, sa_t[p], rm_off)
                    g2 = cp.tile([P, D - 1], F32, name=f"ogg{p}")
                    _gather(nc, g2, og_t[p], og_off)
                    g3 = cp.tile([P, D], F32, name=f"app{p}")
                    _gather(nc, g3, sa_t[p], ap_off)
                    g4 = cp.tile([P, D - 1], F32, name=f"rgg{p}")
                    _gather(nc, g4, sa_t[p], rg_off)
                    rm_t.append(g1)
                    og_g.append(g2)
                    ap_t.append(g3)
                    rg_t.append(g4)

                # act_prev^T stack [16, 128] bf16 per block (rows: hi 0-7, lo 8-15)
                for p in range(2):
                    hi, lo = _split_hi_lo(nc, cp, ap_t[p], f"ap{p}", D)
                    both = cp.tile([P, 2 * D], BF16, name=f"apb{p}")
                    nc.vector.tensor_copy(both[:, bass.ds(0, D, 2)], hi[:])
                    nc.vector.tensor_copy(both[:, bass.ds(1, D, 2)], lo[:])
                    pap = pst.tile([2 * D, P], BF16, name=f"pap{p}", tag="pst")
                    nc.tensor.transpose(pap[:], both[:], identb[:])
                    aps = cp.tile([2 * D, P], BF16, name=f"apstack{p}")
                    nc.vector.tensor_copy(aps[:], pap[:])
                    apstack.append(aps)

                # sg = relu_m * contrib + og_old ; sg[:,7] = ow[:,8]
                cb = cp.tile([14, S], F32, name="cb")
                nc.vector.tensor_copy(cb[:], pe2[:])
                for p in range(2):
                    pct = pst.tile([P, 14], F32, name=f"pct{p}", tag="pst")
                    nc.tensor.transpose(
                        pct[:], cb[:, p * P : (p + 1) * P], ident[0:14, 0:14]
                    )
                    ctr14 = cp.tile([P, 14], F32, name=f"ctr14_{p}")
                    nc.vector.tensor_copy(ctr14[:], pct[:])
                    ctr = cp.tile([P, D - 1], F32, name=f"ctr{p}")
                    nc.vector.tensor_add(
                        ctr[:], ctr14[:, bass.ds(0, D - 1, 2)],
                        ctr14[:, bass.ds(1, D - 1, 2)],
                    )
                    sgA = cp.tile([P, D - 1], F32, name=f"sgA{p}")
                    nc.vector.scalar_tensor_tensor(
                        sgA[:], rm_t[p][:], 0.0, ctr[:],
                        op0=ALU.is_gt, op1=ALU.mult,
                    )
                    sg_p = cp.tile([P, D], F32, name=f"sgp{p}")
                    nc.vector.tensor_add(sg_p[:, 0 : D - 1], sgA[:], og_g[p][:])
                    nc.vector.tensor_copy(sg_p[:, D - 1 : D], ow_t[p][:, D : DP1])
                    nc.scalar.dma_start(o_sg[p], sg_p[:])
                    sg_t.append(sg_p)

                # post-RS: own shard arrives directly; fold hi/lo rows
                hidh = cp.tile([D, S], F32, name="hidh")
                hidl = cp.tile([D, S], F32, name="hidl")
                nc.scalar.dma_start(hidh[:], rs_out[bass.ds(0, D, 2), :])
                nc.scalar.dma_start(hidl[:], rs_out[bass.ds(1, D, 2), :])
                hid_sb = cp.tile([D, S], F32, name="hid_sb")
                nc.vector.tensor_add(hid_sb[:], hidh[:], hidl[:])
                for p in range(2):
                    ph = pst.tile([P, D], F32, name=f"ph{p}", tag="pst")
                    nc.tensor.transpose(
                        ph[:], hid_sb[:, p * P : (p + 1) * P], ident[0:D, 0:D]
                    )
                    sn = cp.tile([P, DP1], F32, name=f"sn{p}")
                    nc.vector.tensor_copy(sn[:, 0:1], x_t[p][:])
                    nc.vector.tensor_relu(sn[:, 1:DP1], ph[:])
                    nc.vector.tensor_copy(sa_t[p][:, bass.ds(t, DP1, MOD)], sn[:])
                    nc.sync.dma_start(o_sa[p], sa_t[p][:])
                    nc.scalar.dma_start(o_state[p], sn[:])
                    sn_t.append(sn)

                # col shards (hi | lo bf16) -> AllGather
                for p in range(2):
                    col_p = cp.tile([P, D], F32, name=f"col{p}")
                    nc.vector.scalar_tensor_tensor(
                        col_p[:, 0 : D - 1], rg_t[p][:], 0.0, sg_t[p][:, 0 : D - 1],
                        op0=ALU.is_gt, op1=ALU.mult,
                    )
                    nc.vector.scalar_tensor_tensor(
                        col_p[:, D - 1 : D], sn_t[p][:, D : DP1], 0.0,
                        ow_t[p][:, D : DP1],
                        op0=ALU.is_gt, op1=ALU.mult,
                    )
                    chi, clo = _split_hi_lo(nc, cp, col_p, f"col{p}", D)
                    colb = cp.tile([P, 2 * D], BF16, name=f"colb{p}")
                    nc.vector.tensor_copy(colb[:, bass.ds(0, D, 2)], chi[:])
                    nc.vector.tensor_copy(colb[:, bass.ds(1, D, 2)], clo[:])
                    nc.scalar.dma_start(ag_in[p * P : (p + 1) * P, :], colb[:])
                nc.gpsimd.collective_compute(
                    "AllGather", ALU.bypass, replica_groups=rg,
                    ins=[ag_in.opt()], outs=[ag_out.opt()],
                )

            # ================= PHASE B: gradients ==========================
            with tc.tile_pool(name="pb", bufs=1) as pb, \
                 tc.tile_pool(name="stg", bufs=3) as stg:
                with tc.tile_pool(name="pst2", bufs=2, space=bass.MemorySpace.PSUM) as pst2:
                    # colT stack [16, 2048] bf16 (rows: hi d=0..7, lo d=0..7)
                    agbig = pb.tile([P, 16 * 2 * D], BF16, name="agbig")
                    agview = ag_out[:].rearrange("(c i) hd -> i c hd", i=P)
                    nc.scalar.dma_start(
                        agbig[:].rearrange("i (c hd) -> i c hd", c=16), agview
                    )
                    ctstack = pb.tile([2 * D, W], BF16, name="ctstack")
                    for c in range(16):
                        pct2 = pst2.tile([2 * D, P], BF16, name=f"pct2_{c}", tag="pst2")
                        nc.tensor.transpose(
                            pct2[:], agbig[:, c * 16 : (c + 1) * 16], identb[:]
                        )
                        nc.vector.tensor_copy(ctstack[:, c * P : (c + 1) * P], pct2[:])

                # gradients: K=4 bf16 outer products
                # lhsT rows (ahi_d, alo_d, ahi_d, alo_d); rhs rows (chi_d, chi_d, clo_d, clo_d)
                lh4 = [[None] * D for _ in range(2)]
                rhs4 = [None] * D
                for d in range(D):
                    r4 = pb.tile([4, W], BF16, name=f"rhs4_{d}")
                    nc.gpsimd.dma_start(r4[0:1, :], ctstack[2 * d : 2 * d + 1, :])
                    nc.gpsimd.dma_start(r4[1:2, :], ctstack[2 * d : 2 * d + 1, :])
                    nc.gpsimd.dma_start(r4[2:3, :], ctstack[2 * d + 1 : 2 * d + 2, :])
                    nc.gpsimd.dma_start(r4[3:4, :], ctstack[2 * d + 1 : 2 * d + 2, :])
                    rhs4[d] = r4
                    for ib in range(2):
                        l4 = pb.tile([4, P], BF16, name=f"lh4_{ib}_{d}")
                        nc.gpsimd.dma_start(l4[0:2, :], apstack[ib][2 * d : 2 * d + 2, :])
                        nc.gpsimd.dma_start(l4[2:4, :], apstack[ib][2 * d : 2 * d + 2, :])
                        lh4[ib][d] = l4

                with tc.tile_pool(name="pg", bufs=2, space=bass.MemorySpace.PSUM) as pgp:
                    for ib in range(2):
                        for wc in range(8):
                            pgt = pgp.tile([P, W], F32, name=f"pg{ib}{wc}", tag="pg")
                            for d in range(D):
                                nc.tensor.matmul(
                                    pgt[:, d * S : (d + 1) * S],
                                    lh4[ib][d][:],
                                    rhs4[d][:, wc * S : (wc + 1) * S],
                                    start=True,
                                    stop=True,
                                )
                            stg_t = stg.tile([P, W], F32, name=f"stg{ib}{wc}", tag="stg")
                            src = pgt[:].rearrange("p (d w) -> p d w", d=D)
                            dst = stg_t[:].rearrange("p (w d) -> p d w", d=D)
                            if wc % 2 == 0:
                                nc.vector.tensor_copy(dst, src)
                            else:
                                nc.scalar.copy(dst, src)
                            nc.sync.dma_start(
                                o_gr[ib, :, wc * W : (wc + 1) * W], stg_t[:]
                            )

    nc.compile()
    return nc


_CACHE = {}


def _get(time):
    t = int(time)
    if t not in _CACHE:
        _CACHE[t] = _build(t)
    return _CACHE[t]


def _prep_in_maps(x, weights, output_weights, state, stored_activations,
                  stored_gradiets, output_gradient):
    x = np.ascontiguousarray(np.asarray(x, dtype=np.float32))
    weights = np.asarray(weights, dtype=np.float32)
    output_weights = np.ascontiguousarray(np.asarray(output_weights, dtype=np.float32))
    state = np.ascontiguousarray(np.asarray(state, dtype=np.float32))
    stored_activations = np.ascontiguousarray(np.asarray(stored_activations, dtype=np.float32))
    stored_gradiets = np.ascontiguousarray(np.asarray(stored_gradiets, dtype=np.float32))
    output_gradient = np.ascontiguousarray(np.asarray(output_gradient, dtype=np.float32))

    ident = np.eye(P, dtype=np.float32)
    sgin = np.ascontiguousarray(stored_gradiets[:, 1:D]).reshape(16, P, D - 1)

    whi = weights.astype(NPBF)
    wlo = (weights - whi.astype(np.float32)).astype(NPBF)

    in_maps = []
    for m in range(NCORES):
        sl = slice(m * S, (m + 1) * S)
        slabh = np.ascontiguousarray(np.transpose(whi[sl], (0, 2, 1))).reshape(2, P, W * D)
        slabl = np.ascontiguousarray(np.transpose(wlo[sl], (0, 2, 1))).reshape(2, P, W * D)
        slabTh = np.ascontiguousarray(
            np.transpose(whi[sl, :, 1:D], (1, 2, 0))
        ).reshape(16, P, S * (D - 1))
        slabTl = np.ascontiguousarray(
            np.transpose(wlo[sl, :, 1:D], (1, 2, 0))
        ).reshape(16, P, S * (D - 1))
        in_maps.append({
            "slabh": slabh,
            "slabl": slabl,
            "slabTh": slabTh,
            "slabTl": slabTl,
            "stt": np.ascontiguousarray(state[sl, 0:D]).reshape(2, P, D),
            "xin": np.ascontiguousarray(x[sl]).reshape(2, P, 1),
            "ow": output_weights[sl].reshape(2, P, DP1),
            "sgin": sgin,
            "ident": ident,
            "sa": stored_activations[sl].reshape(2, P, CT),
            "og": output_gradient[sl].reshape(2, P, CT),
        })
    return in_maps


def _assemble(results, output_weights):
    state = np.concatenate([r["o_state"].reshape(S, DP1) for r in results])
    sg = np.concatenate([r["o_sg"].reshape(S, D) for r in results])
    sa = np.concatenate([r["o_sa"].reshape(S, DP1, MOD) for r in results])
    og = np.concatenate([r["o_og"].reshape(S, DP1, MOD) for r in results])
    grads = np.concatenate([r["o_gr"].reshape(S, W, D) for r in results])
    output_weights = np.asarray(output_weights, dtype=np.float32)
    output = np.float32(np.sum(output_weights * state, dtype=np.float64))
    owg = state.copy()
    return (np.asarray(output, dtype=np.float32), state, sa, sg, og, grads, owg)


def _run(inputs, trace=False):
    time = int(inputs["time"])
    nc = _get(time)
    in_maps = _prep_in_maps(
        inputs["x"], inputs["weights"], inputs["output_weights"], inputs["state"],
        inputs["stored_activations"], inputs["stored_gradiets"],
        inputs["output_gradient"],
    )
    res = bass_utils.run_bass_kernel_spmd(
        nc, in_maps, core_ids=list(range(NCORES)), trace=trace
    )
    outs = _assemble(res.results, inputs["output_weights"])
    return outs, res


def kernel(**inputs):
    outs, _ = _run(inputs, trace=False)
    return outs
